# revision 21
# baseline (speedup 1.0000x reference)
"""Trainium2 Bass kernel for the FERMI fairness-regularizer loss.

Math (see reference):
    h   = relu(X @ fc1_w.T + fc1_b)              [B, H]
    yh  = sigmoid(h @ fc2_w.T + fc2_b)[:, 0]     [B]
    out = LAM * (-sum(yh^2)*||W||_F^2 + 2*sum(yh * (S @ P @ W)) - B) / B

Strategy (pure data parallel over batch, 8 cores):
  - The D=784 contraction is folded host-side to D'=768 = 6*128: with
    A = (|fc2_w| folded fc1) split as [A_main | A_tail] ([768|16] rows of
    A.T), solve A_main.T M = A_tail.T (exact: rank(A_main)=100 <= 768) and
    stream X' = X_main + X_tail @ M.T instead of X.  mm1 is then exactly
    three fp8 DoubleRow passes (256 contraction rows each) per tile — no
    16-row tail pass burning a full 512-cycle PE stream for 2% of the MACs.
  - Host pre-transposes X' so contraction lands on SBUF partitions and
    casts X' and fc1 weights to fp8-e4m3 (fc1 scaled by 2**12 into e4m3's
    normal range; the sigmoid's input scale undoes it for free). PSUM
    accumulation stays fp32; rel err ~1e-6 on the final scalar.
  - fc2_w is folded directly into fc1 rows (weights c_h*fc1_h) and the
    hidden units are permuted so positive-c units occupy partitions
    [0:SPLIT) and negative-c units [SPLIT:HP): with z' = (c*fc1)@x + c*b,
    c*relu(fc1@x+b) equals max(z', 0) for c > 0 and min(z', 0) for c < 0.
    The second-layer contraction is then a plain unweighted partition-sum
    done on the otherwise-idle GPSIMD engine (tensor_reduce axis=C),
    freeing a full 512-cycle PE pass per tile: the PE runs only the three
    DoubleRow mm1 passes.
  - Each core computes yh for its 16384 samples; the three batch sums
    (the "all-reduce" of the sharding hint) are done on host in float64.
  - Pipeline: X packed host-side per load-unit so every DMA is one long
    per-partition run; tapered units (512 head, 512->256 drain, 1024
    mid-stream); const loads ride the ACT HWDGE ring so the X stream owns
    the SP ring.  mm2 for tile k is emitted after tile k+1's mm1 passes
    (PE executes matmuls in strict program order, so lagging mm2 one tile
    hides its wait on the DVE relu); sigmoid + yh writeback are per
    512-tile on the ACT engine.
"""

import sys

try:
    import concourse  # noqa: F401
except ImportError:
    sys.path.insert(0, "/opt/trn_rl_repo")

import ml_dtypes
import numpy as np

import concourse.bass as bass  # noqa: F401  (Bass types used via bacc/tile)
import concourse.tile as tile
from concourse import bacc, bass_isa, mybir
from concourse.bass_utils import run_bass_kernel_spmd

# Problem constants (hardcoded per contract)
B, D, H = 131072, 784, 100
HP = 128                    # H padded to the full PE column width
SPLIT = 64                  # partition split: [0:SPLIT) positive-c units,
                            # [SPLIT:HP) negative-c units (64-aligned as
                            # PSUM partition ranges require)
LAM = 0.1
N_CORES = 8
BS = B // N_CORES           # 16384 samples per core
PD, ND = 128, 6             # 768 = 6 * 128 contraction chunks (tail folded)
DM = PD * ND                # 768 folded contraction dims
F_MM = 512                  # matmul moving free dim (one PSUM bank fp32)
F_BIG = 1024                # samples per DMA super-tile

_BF16 = ml_dtypes.bfloat16
_F8 = ml_dtypes.float8_e4m3     # == TRN float8e4 (max normal 240)
K_SCALE = 12                    # fc1 weights scaled by 2**12 into e4m3 range

_compiled_nc = None


def _unit_lens():
    head = [F_MM] * 4
    tail_t = [F_MM] * 7 + [F_MM // 2, F_MM // 2]
    mid = (BS - sum(head) - sum(tail_t)) // F_BIG
    lens = head + [F_BIG] * mid + tail_t
    assert sum(lens) == BS
    return lens


def _build_bass(reps: int = 1):
    """Per-core SPMD program. Identical on all 8 cores (no collectives).

    reps > 1 wraps the identical kernel body in a hardware loop (used by
    the timing harness to amortize dispatch overhead; the graded kernel
    uses reps=1).
    """
    nc = bacc.Bacc("TRN2", target_bir_lowering=False, debug=False,
                   num_devices=N_CORES)
    f32, bf16 = mybir.dt.float32, mybir.dt.bfloat16

    # X' packed host-side per load-unit: partition p holds each unit's
    # [6 x unit_len] block contiguously, so every unit DMA is a single
    # long per-partition run at full 128-partition port width.
    f8 = mybir.dt.float8e4
    xt = nc.dram_tensor("xt", [PD, ND * BS], f8, kind="ExternalInput")
    a_t = nc.dram_tensor("a_t", [PD, ND, HP], f8, kind="ExternalInput")
    # b12[:, 0] = scaled folded fc1 bias (permuted/padded); b12[0, 1] = fc2 bias
    b12 = nc.dram_tensor("b12", [HP, 2], f32, kind="ExternalInput")
    yh = nc.dram_tensor("yh", [1, BS], f32, kind="ExternalOutput")

    with tile.TileContext(nc) as tc:
        with (
            tc.tile_pool(name="consts", bufs=1) as consts,
            tc.tile_pool(name="xpool", bufs=6) as xpool,
            tc.tile_pool(name="gpool", bufs=4) as gpool,
            tc.tile_pool(name="ypool", bufs=1) as ypool,
            tc.tile_pool(name="yppool", bufs=3) as yppool,
            tc.tile_pool(name="hpsum", bufs=8, space="PSUM") as hpsum,
        ):
            def emit_body():
                # Load units: 1024-sample DMAs (~0.8 MiB) in the middle for
                # low per-DMA overhead; tapered smaller units at both ends so
                # the pipeline fills sooner and drains with less work after
                # the last byte.
                lens = _unit_lens()
                units, pos = [], 0
                for ln in lens:
                    units.append((pos, ln))
                    pos += ln

                def load_unit(off, ln):
                    x_sb = xpool.tile([PD, ND, ln], f8, tag="x_sb")
                    nc.sync.dma_start(
                        out=x_sb.rearrange("p n f -> p (n f)"),
                        in_=xt[:, ND * off:ND * (off + ln)])
                    return x_sb

                # First big X DMA goes out before anything else on the SP
                # HWDGE ring; const loads ride the ACT ring so they overlap.
                x_first = load_unit(*units[0])

                a_sb = consts.tile([PD, ND, HP], f8, tag="a_sb")
                nc.scalar.dma_start(out=a_sb[:], in_=a_t[:])
                b12_sb = consts.tile([HP, 2], f32, tag="b12_sb")
                nc.scalar.dma_start(out=b12_sb[:], in_=b12[:])
                b1_sb = b12_sb[:, 0:1]
                b2_sb = b12_sb[0:1, 1:2]

                yh_sb = ypool.tile([1, BS], f32, tag="yh_sb")

                for iu, (u_off, u_len) in enumerate(units):
                    x_sb = x_first if iu == 0 else load_unit(u_off, u_len)
                    g_t = gpool.tile([HP, F_BIG], bf16, tag="g")
                    n_sub = max(1, u_len // F_MM)
                    for f in range(n_sub):
                        fo = f * F_MM
                        ln = min(F_MM, u_len)
                        hp_t = hpsum.tile([HP, F_MM], f32, tag="hp")
                        hp = hp_t[:, :ln]
                        for dc in range(ND // 2):
                            # DoubleRow: contract 256 d-rows (two 128-row
                            # chunks) per pass, 2 fp8 weights per PE cell.
                            # The PE runs ONLY these passes.
                            nc.tensor.matmul(
                                hp[:],
                                lhsT=a_sb[:, 2 * dc:2 * dc + 2, :],
                                rhs=x_sb[:, 2 * dc:2 * dc + 2,
                                         fo:fo + ln],
                                start=(dc == 0),
                                stop=(dc == ND // 2 - 1),
                                perf_mode=mybir.MatmulPerfMode.DoubleRow,
                            )
                        # positive-c units: g = max(z'+b', 0) = |c|*relu;
                        # negated negative-c units: g = min(z'+b', 0)
                        #   = -|c|*relu.  Pad rows are exactly zero either
                        # way (zero weights + zero bias).
                        nc.vector.tensor_scalar(
                            out=g_t[:SPLIT, fo:fo + ln], in0=hp[:SPLIT, :],
                            scalar1=b1_sb[:SPLIT], scalar2=0.0,
                            op0=mybir.AluOpType.add, op1=mybir.AluOpType.max,
                        )
                        nc.vector.tensor_scalar(
                            out=g_t[SPLIT:, fo:fo + ln], in0=hp[SPLIT:, :],
                            scalar1=b1_sb[SPLIT:], scalar2=0.0,
                            op0=mybir.AluOpType.add, op1=mybir.AluOpType.min,
                        )
                    # yp = sum over the HP partitions (signed sum of unit
                    # contributions) on the otherwise-idle GPSIMD engine.
                    yp_t = yppool.tile([HP, F_BIG], f32, tag="yp")
                    nc.gpsimd.partition_all_reduce(
                        yp_t[:, :u_len], g_t[:, :u_len],
                        channels=HP, reduce_op=bass_isa.ReduceOp.add,
                    )
                    nc.scalar.activation(
                        out=yh_sb[:, u_off:u_off + u_len],
                        in_=yp_t[0:1, :u_len],
                        func=mybir.ActivationFunctionType.Sigmoid,
                        bias=b2_sb[:], scale=float(2.0 ** -K_SCALE),
                    )
                    if u_off + u_len in (BS // 4, BS // 2, 3 * BS // 4):
                        q = u_off + u_len
                        nc.scalar.dma_start(out=yh[:, q - BS // 4:q],
                                            in_=yh_sb[:, q - BS // 4:q])
                # X stream is finished by now — the SP HWDGE ring is idle, so
                # the final output rides it instead of queueing behind the
                # ACT ring's last sigmoid dispatch.
                nc.sync.dma_start(out=yh[:, 3 * BS // 4:],
                                  in_=yh_sb[:, 3 * BS // 4:])

            if reps == 1:
                emit_body()
            else:
                with tc.For_i(0, reps):
                    emit_body()
    nc.compile()
    return nc


def _get_nc():
    global _compiled_nc
    if _compiled_nc is None:
        _compiled_nc = _build_bass()
    return _compiled_nc


def _pack_inputs(X, S, W, fc1_w, fc1_b, fc2_w, fc2_b, P):
    """Host-side prep: fold fc2_w into fc1 rows, permute units by sign of
    c into the fixed SPLIT layout, fold the 16 tail dims into the 768 main
    dims (exact), transpose/pack/cast X."""
    c = np.asarray(fc2_w, np.float64)[0]                  # [H]
    pos_idx = np.flatnonzero(c > 0)
    neg_idx = np.flatnonzero(c < 0)                       # c == 0 units drop
    assert len(pos_idx) <= SPLIT and len(neg_idx) <= HP - SPLIT, (
        len(pos_idx), len(neg_idx))
    # slot assignment: positives at [0:npos), negated negatives at
    # [SPLIT:SPLIT+nneg); all other slots stay exactly zero
    slots = np.concatenate([pos_idx, neg_idx])
    slot_of = list(range(len(pos_idx))) + [
        SPLIT + i for i in range(len(neg_idx))]

    A = np.asarray(fc1_w, np.float64) * c[:, None]        # [H, D] (c folded)
    # Fold tail: solve A_main.T M = A_tail.T exactly (rank H <= DM), then
    # X' = X_main + X_tail @ M.T reproduces A.T X with a DM-dim contraction.
    A_main, A_tail = A[:, :DM], A[:, DM:]                 # [H,768], [H,16]
    M = np.linalg.lstsq(A_main, A_tail, rcond=None)[0]    # [768, 16]
    resid = np.abs(A_main @ M - A_tail).max()
    assert resid < 1e-8 * max(1.0, np.abs(A_tail).max()), resid

    AT = A_main.T * (2.0 ** K_SCALE)                      # [768, H]
    AT = np.clip(AT, -240.0, 240.0).astype(_F8)
    ATp = np.zeros((DM, HP), _F8)
    ATp[:, slot_of] = AT[:, slots]
    # [p, n, h]: per-partition contiguous weight DMA
    a_t = np.ascontiguousarray(
        ATp.reshape(ND, PD, HP).transpose(1, 0, 2))
    b1 = np.asarray(fc1_b, np.float64) * c * (2.0 ** K_SCALE)
    b12_v = np.zeros((HP, 2), np.float32)
    b12_v[slot_of, 0] = b1[slots].astype(np.float32)
    b12_v[0, 1] = np.float32(np.asarray(fc2_b, np.float32).reshape(-1)[0])

    Xf = np.asarray(X, np.float32)
    Xp = Xf[:, :DM] + Xf[:, DM:] @ M.T.astype(np.float32)  # [B, 768]
    Xb = np.clip(Xp, -240.0, 240.0).astype(_F8)
    lens = _unit_lens()
    in_maps = []
    for core in range(N_CORES):
        xs = Xb[core * BS:(core + 1) * BS]                # [BS, 768]
        # per-unit-contiguous: xt[p, ND*off + n*ln + col]
        #   = X'[off + col, n*128 + p]
        xp = np.empty((PD, ND * BS), _F8)
        off = 0
        for ln in lens:
            blk = xs[off:off + ln].reshape(ln, ND, PD)
            xp[:, ND * off:ND * (off + ln)] = (
                blk.transpose(2, 1, 0).reshape(PD, ND * ln))
            off += ln
        in_maps.append({"xt": xp, "a_t": a_t, "b12": b12_v})
    return in_maps


def kernel(X, S, W, fc1_w, fc1_b, fc2_w, fc2_b, P):
    nc = _get_nc()
    in_maps = _pack_inputs(X, S, W, fc1_w, fc1_b, fc2_w, fc2_b, P)
    res = run_bass_kernel_spmd(nc, in_maps, core_ids=list(range(N_CORES)))
    yh = np.concatenate([r["yh"][0] for r in res.results]).astype(np.float64)

    S64 = np.asarray(S, np.float64)
    W64 = np.asarray(W, np.float64)
    PW = np.asarray(P, np.float64) @ W64                  # [2, 1]
    tr_wwt = float(np.sum(W64 * W64))
    summation = (-float(yh @ yh) * tr_wwt
                 + 2.0 * float((yh @ S64) @ PW[:, 0])
                 - B)
    return np.float32(LAM * summation / B)
